# revision 80
# baseline (speedup 1.0000x reference)
"""Trainium2 Bass kernel for nn_PretrainedFeatureExtractor.

Data-parallel over batch: 8 point clouds -> 8 NeuronCores, one cloud each.
BatchNorm statistics are EXACT global batch statistics:
  - bn1: affine computed on host from the 2nd moments of the raw input
    (x is 3-dim, so mean/var of l1_w@x come from 12 numbers).
  - bn2/ebn1/ebn2/pbn: per-device partial (mean, E[x^2]) AllReduced
    across the 8 cores (tiny [128,2] messages), then exact affine.
  - gbn1/gbn2: gin [1,128] AllGathered; every core redundantly computes
    the tiny global MLP on the full 8-row batch (exact stats).
  - p-layer: glob enters as a per-device constant vector pvec; it is
    folded into the partial stats and the ACT bias (per-partition).

Per-device pipeline (feature-major layout [feat, point]):
  L1/L2 encoder -> local_T [128, 2048]
  s'[i,j] = <x_i,x_j> - 0.5|x_j|^2  (ranking-equivalent to -dist); per
  score tile all 4 main matmuls issue before the sqF-dependent ones so
  the in-order PE queue never blocks on sqF
  top-16 = top-16 of the union of top-8 of each of 8 column chunks
  (chunk max8 + merge via match_replace + two full-row max_index scans)
  gpsimd ap_gather of neighbor features + DVE max-reduce, lagged by one
  group so the in-order DVE never stalls on an in-flight gather -> edge_T
  e1/e2 encoder -> enh_T ; gin ; p-layer matmuls + raw stats run before
  the serial global-MLP chain and overlap it (pvec folded into the
  partials afterwards) ; AllReduce -> affine -> relu ; PE transpose to
  point-major (DVE does the psum->sbuf copies; out-DMAs split 12:4 over
  the ACT/SP queues) ; + 0.1*cat_bias via K=1 matmul ; DMA out.

  All heavy matmuls run as float32r (1 cyc/row in the cost model vs 4 for
  fp32); f32r rounds operands to ~12 mantissa bits, which costs ~1.1e-2
  rel err (budget 2e-2) via occasional KNN neighbor-rank flips.
"""
import sys

sys.path.insert(0, "/opt/trn_rl_repo")

from contextlib import ExitStack

import numpy as np

from concourse import bacc, bass, bass_isa, library_config, mybir, tile
from concourse.bass_utils import run_bass_kernel_spmd

N = 2048
NT = N // 128          # 16 row tiles
f32r_dt = mybir.dt.float32r

def _r(ap):
    """Bitcast an fp32 AP to float32r for 4x-faster PE (1 cyc/row at N>=256).
    No-op if already f32r."""
    if ap.dtype == f32r_dt:
        return ap
    return ap.bitcast(f32r_dt)
K = 16
NCH = 8                # top-k column chunks
CHW = N // NCH         # 256
GCH = 2                # row-tiles per gather chunk
NCORES = 8
EPS = 1e-5

f32 = mybir.dt.float32
f32r = mybir.dt.float32r
i16 = mybir.dt.int16
u16 = mybir.dt.uint16
RG = [list(range(NCORES))]

_CACHE = {}


def _partial_stats(nc, sb_small, psum_ap, nfree, pvec=None):
    """DVE partial stats of psum y0: returns sbuf [128, 2] = (mean, E[y^2])
    where y = y0 + pvec (pvec optional per-partition column)."""
    ngroups = nfree // 512
    st = sb_small.tile([128, ngroups * 6], f32, tag="bnst")
    for g in range(ngroups):
        nc.vector.bn_stats(st[:, g * 6:(g + 1) * 6],
                           psum_ap[:, g * 512:(g + 1) * 512])
    ag = sb_small.tile([128, 2], f32, tag="bnag")
    nc.vector.bn_aggr(ag[:], st[:])
    pm = sb_small.tile([128, 2], f32, tag="bnpm")
    if pvec is not None:
        nc.vector.tensor_tensor(out=pm[:, 0:1], in0=ag[:, 0:1], in1=pvec,
                                op=mybir.AluOpType.add)
    else:
        nc.vector.tensor_copy(pm[:, 0:1], ag[:, 0:1])
    m2 = sb_small.tile([128, 1], f32, tag="bnm2")
    nc.vector.tensor_tensor(out=m2[:], in0=pm[:, 0:1], in1=pm[:, 0:1],
                            op=mybir.AluOpType.mult)
    nc.vector.tensor_tensor(out=pm[:, 1:2], in0=ag[:, 1:2], in1=m2[:],
                            op=mybir.AluOpType.add)
    return pm


def _affine_from_sums(nc, sb_small, sums_ap, g_col, b_col, pvec=None,
                      tagsfx=""):
    """From AllReduced sums [128,2] -> (a, b'') per-partition columns.

    m = sums0/8 ; Ex2 = sums1/8 ; var = Ex2 - m^2
    a = g * rsqrt(var+eps) ; b'' = b - m*a (+ a*pvec if pvec given)
    """
    m = sb_small.tile([128, 1], f32, tag="afm")
    nc.scalar.mul(m[:], sums_ap[:, 0:1], 1.0 / NCORES)
    ex2 = sb_small.tile([128, 1], f32, tag="afex")
    nc.scalar.mul(ex2[:], sums_ap[:, 1:2], 1.0 / NCORES)
    mm = sb_small.tile([128, 1], f32, tag="afmm")
    nc.vector.tensor_tensor(out=mm[:], in0=m[:], in1=m[:],
                            op=mybir.AluOpType.mult)
    var = sb_small.tile([128, 1], f32, tag="afv")
    nc.vector.tensor_tensor(out=var[:], in0=ex2[:], in1=mm[:],
                            op=mybir.AluOpType.subtract)
    ve = sb_small.tile([128, 1], f32, tag="afve")
    nc.vector.tensor_scalar(out=ve[:], in0=var[:], scalar1=EPS, scalar2=None,
                            op0=mybir.AluOpType.add)
    sd = sb_small.tile([128, 1], f32, tag="afsd")
    nc.scalar.sqrt(sd[:], ve[:])
    rs = sb_small.tile([128, 1], f32, tag="afrs")
    nc.vector.reciprocal(rs[:], sd[:])
    a = sb_small.tile([128, 1], f32, tag="afa" + tagsfx)
    nc.vector.tensor_tensor(out=a[:], in0=rs[:], in1=g_col,
                            op=mybir.AluOpType.mult)
    if pvec is not None:
        d = sb_small.tile([128, 1], f32, tag="afd")
        nc.vector.tensor_tensor(out=d[:], in0=pvec, in1=m[:],
                                op=mybir.AluOpType.subtract)
        m = d
    # b'' = b + (pvec - m)*a   (pvec=0 -> b - m*a)
    ma = sb_small.tile([128, 1], f32, tag="afma")
    nc.vector.tensor_tensor(out=ma[:], in0=m[:], in1=a[:],
                            op=mybir.AluOpType.mult)
    bp = sb_small.tile([128, 1], f32, tag="afbp" + tagsfx)
    if pvec is not None:
        nc.vector.tensor_tensor(out=bp[:], in0=b_col, in1=ma[:],
                                op=mybir.AluOpType.add)
    else:
        nc.vector.tensor_tensor(out=bp[:], in0=b_col, in1=ma[:],
                                op=mybir.AluOpType.subtract)
    return a, bp


def build_bass(mock_cc=False):
    nc = bacc.Bacc(None, target_bir_lowering=False, debug=False,
                   num_devices=NCORES)

    def din(name, shape, dt=f32):
        return nc.dram_tensor(name, shape, dt, kind="ExternalInput")

    xT_d = din("xT", [3, N], f32r)
    l1_wT_d = din("l1_wT", [3, 64], f32r)
    bn1ab_d = din("bn1ab", [64, 2])      # host-computed exact affine
    l2_wT_d = din("l2_wT", [64, 128], f32r)
    e1_wT_d = din("e1_wT", [256, 128], f32r)
    e2_wT_d = din("e2_wT", [128, 128], f32r)
    p_wT_d = din("p_wT", [128, 512], f32r)   # p_w[:, :128].T
    pg_wT_d = din("pg_wT", [512, 512])   # p_w[:, 128:640].T
    g1_wT_d = din("g1_wT", [128, 256])   # g1_w.T
    g2_wT_d = din("g2_wT", [256, 512])   # g2_w.T
    bn2_d = din("bn2", [128, 2])
    ebn1_d = din("ebn1", [128, 2])
    ebn2_d = din("ebn2", [128, 2])
    gbn1_d = din("gbn1", [128, 4])       # [g blk0, g blk1, b blk0, b blk1]
    gbn2_d = din("gbn2", [128, 8])       # [g f0..3, b f0..3]
    pbn_d = din("pbn", [128, 8])         # [gamma f0..3, beta f0..3]
    catb_d = din("catb", [1, 512], f32r)  # -0.2 * cat_bias[cat_id]
    ident_d = din("ident", [128, 128])
    neghalf_d = din("neghalf", [128, 128], f32r)
    neghalfb_d = din("neghalfb", [128, 128], mybir.dt.bfloat16)
    repmat_d = din("repmat", [16, 128])

    out_d = nc.dram_tensor("out", [N, 512], f32, kind="ExternalOutput")

    with tile.TileContext(nc) as tc, ExitStack() as ctx:
        sbW = ctx.enter_context(tc.tile_pool(name="sbW", bufs=1))
        sbA = ctx.enter_context(tc.tile_pool(name="sbA", bufs=1))
        sbS = ctx.enter_context(tc.tile_pool(name="sbS", bufs=3))
        sb_small = ctx.enter_context(tc.tile_pool(name="sbsm", bufs=2))
        sbG = ctx.enter_context(tc.tile_pool(name="sbG", bufs=3))
        sbF = ctx.enter_context(tc.tile_pool(name="sbF", bufs=4))
        sbO = ctx.enter_context(tc.tile_pool(name="sbO", bufs=3))
        dramP = ctx.enter_context(tc.tile_pool(name="dram", bufs=2,
                                               space="DRAM"))
        psB = ctx.enter_context(tc.tile_pool(name="psB", bufs=1, space="PSUM"))
        psS = ctx.enter_context(tc.tile_pool(name="psS", bufs=4, space="PSUM"))

        # ---------------- load weights/constants ----------------
        _eng = [nc.sync, nc.scalar]
        _ei = [0]

        def wtile(dram, shape, dt=f32):
            t = sbW.tile(shape, dt, tag=dram.name)
            _eng[_ei[0] % 2].dma_start(t[:], dram[:])
            _ei[0] += 1
            return t

        xT = wtile(xT_d, [3, N], f32r)
        l1_wT = wtile(l1_wT_d, [3, 64], f32r)
        bn1ab = wtile(bn1ab_d, [64, 2])
        l2_wT = wtile(l2_wT_d, [64, 128], f32r)
        bn2 = wtile(bn2_d, [128, 2])
        neghalf = wtile(neghalf_d, [128, 128], f32r)
        ident = wtile(ident_d, [128, 128])
        repmat = wtile(repmat_d, [16, 128])
        e1_wTa = sbW.tile([128, 128], f32r, tag="e1a")
        nc.sync.dma_start(e1_wTa[:], e1_wT_d[0:128, :])
        e1_wTb = sbW.tile([128, 128], f32, tag="e1b")
        nc.sync.dma_start(e1_wTb[:], e1_wT_d[128:256, :].bitcast(f32))
        e2_wT = wtile(e2_wT_d, [128, 128], f32r)
        p_wT = wtile(p_wT_d, [128, 512], f32r)
        pg_wT = [sbW.tile([128, 512], f32, tag="pg%d" % i,
                          name="pg_wT%d" % i) for i in range(4)]
        for i in range(4):
            nc.sync.dma_start(pg_wT[i][:], pg_wT_d[i * 128:(i + 1) * 128, :])
        g1_wT = wtile(g1_wT_d, [128, 256])
        g2_wT = [sbW.tile([128, 512], f32, tag="g2%d" % i,
                          name="g2_wT%d" % i) for i in range(2)]
        for i in range(2):
            nc.sync.dma_start(g2_wT[i][:], g2_wT_d[i * 128:(i + 1) * 128, :])
        ebn1 = wtile(ebn1_d, [128, 2])
        ebn2 = wtile(ebn2_d, [128, 2])
        gbn1 = wtile(gbn1_d, [128, 4])
        gbn2 = wtile(gbn2_d, [128, 8])
        pbn = wtile(pbn_d, [128, 8])
        catb = wtile(catb_d, [1, 512], f32r)
        neghalfb = wtile(neghalfb_d, [128, 128], mybir.dt.bfloat16)

        nc.gpsimd.load_library(library_config.ap_gather)

        def all_reduce(pm, width):
            """AllReduce sbuf [128, width] partial -> sbuf sums [128, width]."""
            if mock_cc:
                sums = sb_small.tile([128, width], f32, tag="ccs",
                                     name="sums_m%d" % nc.next_id())
                nc.vector.tensor_scalar(out=sums[:], in0=pm[:],
                                        scalar1=float(NCORES), scalar2=None,
                                        op0=mybir.AluOpType.mult)
                return sums
            cin = dramP.tile([128, width], f32, tag="cc_in")
            cout = dramP.tile([128, width], f32, tag="cc_out")
            nc.sync.dma_start(cin[:], pm[:])
            nc.gpsimd.collective_compute(
                "AllReduce", mybir.AluOpType.add, replica_groups=RG,
                ins=[cin.opt()], outs=[cout.opt()])
            sums = sb_small.tile([128, width], f32, tag="ccs")
            nc.sync.dma_start(sums[:], cout[:])
            return sums

        # ---------------- local encoder ----------------
        h1 = sbA.tile([64, N], f32r)
        ps = psB.tile([128, N], f32, tag="big")
        for j in range(0, N, 512):
            nc.tensor.matmul(ps[:64, j:j + 512], _r(l1_wT[:]),
                             _r(xT[:, j:j + 512]),
                             start=True, stop=True)
        nc.scalar.activation(h1[:], ps[:64, :],
                             mybir.ActivationFunctionType.Relu,
                             bias=bn1ab[:, 1:2], scale=bn1ab[:, 0:1])

        localT = sbA.tile([128, N], f32r)
        ps = psB.tile([128, N], f32, tag="big")
        for j in range(0, N, 512):
            nc.tensor.matmul(ps[:, j:j + 512], _r(l2_wT[:]),
                             _r(h1[:, j:j + 512]),
                             start=True, stop=True)
        pm = _partial_stats(nc, sb_small, ps[:], N)
        sums = all_reduce(pm, 2)
        a2, b2 = _affine_from_sums(nc, sb_small, sums[:], bn2[:, 0:1],
                                   bn2[:, 1:2])
        nc.scalar.activation(localT[:], ps[:],
                             mybir.ActivationFunctionType.Relu,
                             bias=b2[:], scale=a2[:])

        sqF = sbA.tile([128, N], f32r)
        nc.scalar.square(sqF[:], localT[:])

        # ---------------- knn top-16 ----------------
        # edgeT/g_out stay f32: DVE tensor_reduce / Pool ap_gather producing
        # f32r is unproven on HW; the e1b matmul runs as plain f32 instead.
        edgeT = sbA.tile([128, N], f32)
        CW = GCH * 128               # wrapped columns per chunk

        def issue_gather(wr_ap, ncols, name):
            rep = psS.tile([128, ncols], f32, tag="sm", name="rep" + name)
            nc.tensor.matmul(rep[:], repmat[:], wr_ap,
                             start=True, stop=True)
            w_i16 = sbG.tile([128, ncols], i16, tag="w16", name="w16_" + name)
            nc.scalar.copy(w_i16[:], rep[:])
            g_out = sbG.tile([128, ncols * 16], f32, tag="gout",
                             name="gout" + name)
            nc.gpsimd.ap_gather(
                out_ap=g_out[:], in_ap=localT[:].bitcast(f32), idxs_ap=w_i16[:],
                channels=128, num_elems=N, d=1, num_idxs=ncols * 16)
            return g_out

        def reduce_gather(off, ncols, g_out):
            nc.vector.tensor_reduce(
                out=edgeT[:, off:off + ncols],
                in_=g_out[:].rearrange("p (n k) -> p n k", k=K),
                axis=mybir.AxisListType.X, op=mybir.AluOpType.max)

        pending = None
        wrapped_f32 = []
        for c in range(NT // GCH):
            wv = sbA.tile([16, GCH * 128], f32, tag="wr%d" % c,
                          name="wrapped%d" % c)
            wrapped_f32.append(wv)
        for t in range(NT):
            sp = psB.tile([128, N], f32, tag="big")
            # all main MMs first: the first neghalf MM waits on sqF, and the
            # in-order PE queue would otherwise block the remaining mains
            for j in range(0, N, 512):
                nc.tensor.matmul(sp[:, j:j + 512],
                                 _r(localT[:, t * 128:(t + 1) * 128]),
                                 _r(localT[:, j:j + 512]),
                                 start=True, stop=False)
            for j in range(0, N, 512):
                nc.tensor.matmul(sp[:, j:j + 512], neghalf[:],
                                 _r(sqF[:, j:j + 512]),
                                 start=False, stop=True)
            s_sb = sbS.tile([128, N], f32, tag="s_sb")
            nc.scalar.copy(s_sb[:], sp[:])

            vals = sb_small.tile([128, NCH * 8], f32, tag="vals")
            for c in range(NCH):
                nc.vector.max(vals[:, c * 8:(c + 1) * 8],
                              s_sb[:, c * CHW:(c + 1) * CHW])
            m1 = sb_small.tile([128, 8], f32, tag="mg1")
            nc.vector.max(m1[:], vals[:])
            vmod = sb_small.tile([128, NCH * 8], f32, tag="vmod")
            nc.vector.match_replace(vmod[:], m1[:], vals[:], -1e30)
            m2 = sb_small.tile([128, 8], f32, tag="mg2")
            nc.vector.max(m2[:], vmod[:])
            idx16 = sb_small.tile([128, K], u16, tag="idx16")
            nc.vector.max_index(idx16[:, 0:8], m1[:], s_sb[:])
            nc.vector.max_index(idx16[:, 8:16], m2[:], s_sb[:])
            idxf = sb_small.tile([128, K], f32, tag="idxf")
            nc.vector.tensor_copy(idxf[:], idx16[:])
            tp = psS.tile([16, 128], f32, tag="sm")
            nc.tensor.transpose(tp[:], idxf[:], ident[:])
            nc.scalar.copy(wrapped_f32[t // GCH][:, (t % GCH) * 128:(t % GCH + 1) * 128], tp[:])
            if t >= NT - GCH:
                # last two tiles gather singly: tile 14's gather+reduce hide
                # under tile 15's top-k, and the post-loop drain halves
                g = issue_gather(
                    wrapped_f32[t // GCH][:, (t % GCH) * 128:
                                          (t % GCH + 1) * 128],
                    128, "s%d" % t)
                if pending is not None:
                    reduce_gather(*pending)
                pending = (t * 128, 128, g)
            elif t % GCH == GCH - 1:
                c = t // GCH
                g = issue_gather(wrapped_f32[c][:], CW, "%d" % c)
                if pending is not None:
                    reduce_gather(*pending)
                pending = (c * CW, CW, g)

        reduce_gather(*pending)

        # ---------------- edge encoder ----------------
        h3 = sbA.tile([128, N], f32r)
        ps = psB.tile([128, N], f32, tag="big")
        for j in range(0, N, 512):
            nc.tensor.matmul(ps[:, j:j + 512], _r(e1_wTa[:]),
                             _r(localT[:, j:j + 512]),
                             start=True, stop=False)
            nc.tensor.matmul(ps[:, j:j + 512], e1_wTb[:],
                             edgeT[:, j:j + 512],
                             start=False, stop=True)
        pm = _partial_stats(nc, sb_small, ps[:], N)
        sums = all_reduce(pm, 2)
        ae1, be1 = _affine_from_sums(nc, sb_small, sums[:], ebn1[:, 0:1],
                                     ebn1[:, 1:2])
        nc.scalar.activation(h3[:], ps[:], mybir.ActivationFunctionType.Relu,
                             bias=be1[:], scale=ae1[:])

        enhT = sbA.tile([128, N], f32r)
        ps = psB.tile([128, N], f32, tag="big")
        for j in range(0, N, 512):
            nc.tensor.matmul(ps[:, j:j + 512], _r(e2_wT[:]),
                             _r(h3[:, j:j + 512]),
                             start=True, stop=True)
        pm = _partial_stats(nc, sb_small, ps[:], N)
        sums = all_reduce(pm, 2)
        ae2, be2 = _affine_from_sums(nc, sb_small, sums[:], ebn2[:, 0:1],
                                     ebn2[:, 1:2])
        nc.scalar.activation(enhT[:], ps[:], mybir.ActivationFunctionType.Relu,
                             bias=be2[:], scale=ae2[:])

        # ---------------- global branch ----------------
        gin = sb_small.tile([128, 1], f32, tag="gin")
        nc.vector.tensor_reduce(out=gin[:], in_=enhT[:].bitcast(f32),
                                axis=mybir.AxisListType.X,
                                op=mybir.AluOpType.max)

        # p-layer matmuls + raw stats + copies run here so they overlap the
        # serial global-MLP chain below; the pvec fold is applied afterwards.
        y0 = []
        praw = []
        for f in range(4):
            ps = psB.tile([128, N], f32, tag="big")
            for j in range(0, N, 512):
                nc.tensor.matmul(
                    ps[:, j:j + 512],
                    _r(p_wT[:, f * 128:(f + 1) * 128]),
                    _r(enhT[:, j:j + 512]),
                    start=True, stop=True)
            yf = sbF.tile([128, N], f32, tag="feat")
            nc.scalar.copy(yf[:], ps[:])
            y0.append(yf)
            st_p = sb_small.tile([128, 24], f32, tag="bnst")
            for g in range(4):
                nc.vector.bn_stats(st_p[:, g * 6:(g + 1) * 6],
                                   yf[:, g * 512:(g + 1) * 512])
            agf = sb_small.tile([128, 2], f32, tag="agp%d" % f)
            nc.vector.bn_aggr(agf[:], st_p[:])
            praw.append(agf)

        ginr = psS.tile([16, 128], f32, tag="sm")
        nc.tensor.transpose(ginr[:1, :], gin[:], ident[:])
        gin_row = sb_small.tile([1, 128], f32, tag="ginrow")
        nc.scalar.copy(gin_row[:], ginr[:1, :])
        gin_all = sb_small.tile([NCORES, 128], f32, tag="ginall")
        if mock_cc:
            for r in range(NCORES):
                nc.sync.dma_start(gin_all[r:r + 1, :], gin_row[:])
        else:
            ag_in = dramP.tile([1, 128], f32, tag="ag_in")
            ag_out = dramP.tile([NCORES, 128], f32, tag="ag_out")
            nc.sync.dma_start(ag_in[:], gin_row[:])
            nc.gpsimd.collective_compute(
                "AllGather", mybir.AluOpType.bypass, replica_groups=RG,
                ins=[ag_in.opt()], outs=[ag_out.opt()])
            nc.sync.dma_start(gin_all[:], ag_out[:])
        ginT = sb_small.tile([128, NCORES], f32, tag="ginT")
        # transpose [8,128] -> [128,8] via matmul with I8
        gps = psS.tile([128, NCORES], f32, tag="sm")
        nc.tensor.matmul(gps[:], gin_all[:], ident[:8, :8],
                         start=True, stop=True)
        nc.scalar.copy(ginT[:], gps[:])

        # global MLP on the full 8-batch (replicated on every core; exact)
        def bn8_relu(nc, y_ps, g_col, b_col, out_ap):
            st = sb_small.tile([128, 6], f32, tag="g8st")
            nc.vector.bn_stats(st[:], y_ps)
            ag = sb_small.tile([128, 2], f32, tag="g8ag")
            nc.vector.bn_aggr(ag[:], st[:])
            ve = sb_small.tile([128, 1], f32, tag="g8ve")
            nc.vector.tensor_scalar(out=ve[:], in0=ag[:, 1:2], scalar1=EPS,
                                    scalar2=None, op0=mybir.AluOpType.add)
            sd = sb_small.tile([128, 1], f32, tag="g8sd")
            nc.scalar.sqrt(sd[:], ve[:])
            rs = sb_small.tile([128, 1], f32, tag="g8rs")
            nc.vector.reciprocal(rs[:], sd[:])
            a = sb_small.tile([128, 1], f32, tag="g8a")
            nc.vector.tensor_tensor(out=a[:], in0=rs[:], in1=g_col,
                                    op=mybir.AluOpType.mult)
            ma = sb_small.tile([128, 1], f32, tag="g8ma")
            nc.vector.tensor_tensor(out=ma[:], in0=ag[:, 0:1], in1=a[:],
                                    op=mybir.AluOpType.mult)
            bp = sb_small.tile([128, 1], f32, tag="g8bp")
            nc.vector.tensor_tensor(out=bp[:], in0=b_col, in1=ma[:],
                                    op=mybir.AluOpType.subtract)
            nc.scalar.activation(out_ap, y_ps,
                                 mybir.ActivationFunctionType.Relu,
                                 bias=bp[:], scale=a[:])
            return a, bp

        # g1: two output blocks of 128 feats, batch=8 cols (+ own col 8)
        gcat = sb_small.tile([128, NCORES + 1], f32, tag="gcat")
        nc.vector.tensor_copy(gcat[:, :NCORES], ginT[:])
        nc.vector.tensor_copy(gcat[:, NCORES:], gin[:])
        h1g = sb_small.tile([128, 2 * (NCORES + 1)], f32, tag="h1g")
        for blk in range(2):
            gp = psS.tile([128, NCORES + 1], f32, tag="sm")
            nc.tensor.matmul(gp[:],
                             g1_wT[:, blk * 128:(blk + 1) * 128],
                             gcat[:], start=True, stop=True)
            a, bp = bn8_relu(nc, gp[:, :NCORES], gbn1[:, blk:blk + 1],
                             gbn1[:, 2 + blk:3 + blk],
                             h1g[:, blk * (NCORES + 1):
                                 (blk + 1) * (NCORES + 1) - 1])
            nc.scalar.activation(h1g[:, (blk + 1) * (NCORES + 1) - 1:
                                     (blk + 1) * (NCORES + 1)],
                                 gp[:, NCORES:],
                                 mybir.ActivationFunctionType.Relu,
                                 bias=bp[:], scale=a[:])
        # g2: four output blocks of 128 feats
        glob_own = sb_small.tile([128, 4], f32, tag="globown")
        for blk in range(4):
            gp = psS.tile([128, NCORES + 1], f32, tag="sm")
            for kb in range(2):
                nc.tensor.matmul(
                    gp[:],
                    g2_wT[kb][:, blk * 128:(blk + 1) * 128],
                    h1g[:, kb * (NCORES + 1):(kb + 1) * (NCORES + 1)]
                        ,
                    start=(kb == 0), stop=(kb == 1))
            gdump = sb_small.tile([128, NCORES], f32, tag="gdump")
            a, bp = bn8_relu(nc, gp[:, :NCORES], gbn2[:, blk:blk + 1],
                             gbn2[:, 4 + blk:5 + blk], gdump[:])
            nc.scalar.activation(glob_own[:, blk:blk + 1], gp[:, NCORES:],
                                 mybir.ActivationFunctionType.Relu,
                                 bias=bp[:], scale=a[:])

        # pvec[f] = p_w[:, 128:640] @ glob_own  (4 feature blocks)
        pvec = sb_small.tile([128, 4], f32, tag="pvec")
        for f in range(4):
            pp = psS.tile([128, 1], f32, tag="sm")
            for gb in range(4):
                nc.tensor.matmul(
                    pp[:],
                    pg_wT[gb][:, f * 128:(f + 1) * 128],
                    glob_own[:, gb:gb + 1],
                    start=(gb == 0), stop=(gb == 3))
            nc.scalar.copy(pvec[:, f:f + 1], pp[:])

        # ---------------- p-layer stats fold + affine ----------------
        pmall = sb_small.tile([128, 8], f32, tag="pmall")
        for f in range(4):
            # (mean, var) + pvec -> (mean+pvec, E[(y+pvec)^2]) partials
            nc.vector.tensor_tensor(out=pmall[:, f * 2:f * 2 + 1],
                                    in0=praw[f][:, 0:1],
                                    in1=pvec[:, f:f + 1],
                                    op=mybir.AluOpType.add)
            m2p = sb_small.tile([128, 1], f32, tag="bnm2")
            nc.vector.tensor_tensor(out=m2p[:], in0=pmall[:, f * 2:f * 2 + 1],
                                    in1=pmall[:, f * 2:f * 2 + 1],
                                    op=mybir.AluOpType.mult)
            nc.vector.tensor_tensor(out=pmall[:, f * 2 + 1:f * 2 + 2],
                                    in0=praw[f][:, 1:2], in1=m2p[:],
                                    op=mybir.AluOpType.add)
        sums = all_reduce(pmall, 8)
        featT = y0
        affs = []
        for f in range(4):
            af, bf = _affine_from_sums(nc, sb_small, sums[:, f * 2:(f + 1) * 2],
                                       pbn[:, f:f + 1], pbn[:, 4 + f:5 + f],
                                       pvec=pvec[:, f:f + 1], tagsfx=str(f))
            affs.append((af, bf))
        # chunked column-major so tile 0's transposes start after 4 small
        # acts instead of 4 full-row ones
        for c in range(4):
            for f in range(4):
                af, bf = affs[f]
                nc.scalar.activation(y0[f][:, c * 512:(c + 1) * 512],
                                     y0[f][:, c * 512:(c + 1) * 512],
                                     mybir.ActivationFunctionType.Relu,
                                     bias=bf[:], scale=af[:])

        # ---------------- transpose + cat bias + out ----------------
        for t in range(NT):
            po = psS.tile([128, 512], f32, tag="sm")
            nc.tensor.matmul(po[:], neghalf[0:1, :],
                             catb[:],
                             start=True, stop=False, skip_group_check=True)
            for f in range(4):
                nc.tensor.matmul(po[:, f * 128:(f + 1) * 128],
                                 featT[f][:, t * 128:(t + 1) * 128],
                                 ident[:], start=False, stop=(f == 3),
                                 is_transpose=True, skip_group_check=True)
            ot = sbO.tile([128, 512], f32, tag="ot")
            nc.vector.tensor_copy(ot[:], po[:])
            _oeng = nc.sync if t % 4 == 3 else nc.scalar
            _oeng.dma_start(out_d[t * 128:(t + 1) * 128, :], ot[:])

    nc.compile()
    return nc


def host_prep(inputs):
    """Build per-core in_maps from the full inputs."""
    pc = np.ascontiguousarray(np.asarray(inputs["point_cloud"], np.float32))
    B = pc.shape[0]
    cat_ids = np.asarray(inputs["category_ids"]).astype(np.int64)
    catb_all = np.asarray(inputs["cat_bias"], np.float32)

    # exact bn1 affine from input moments (x is 3-dim)
    x = pc.reshape(-1, 3).astype(np.float64)
    mu = x.mean(0)
    cov = (x.T @ x) / x.shape[0] - np.outer(mu, mu)   # biased
    w1 = np.asarray(inputs["l1_w"], np.float64)
    b1lin = np.asarray(inputs["l1_b"], np.float64)
    m1 = w1 @ mu + b1lin
    v1 = np.einsum("fi,ij,fj->f", w1, cov, w1)
    a1 = np.asarray(inputs["bn1_g"], np.float64) / np.sqrt(v1 + EPS)
    b1 = np.asarray(inputs["bn1_b"], np.float64) - m1 * a1

    def T(name):
        return np.ascontiguousarray(np.asarray(inputs[name], np.float32).T)

    common = {
        "l1_wT": T("l1_w"),
        "bn1ab": np.stack([a1, b1], 1).astype(np.float32),
        "l2_wT": T("l2_w"),
        "e1_wT": T("e1_w"),
        "e2_wT": T("e2_w"),
        "p_wT": np.ascontiguousarray(
            np.asarray(inputs["p_w"], np.float32)[:, :128].T),
        "pg_wT": np.ascontiguousarray(
            np.asarray(inputs["p_w"], np.float32)[:, 128:].T),
        "g1_wT": T("g1_w"),
        "g2_wT": T("g2_w"),
        "bn2": np.stack([np.asarray(inputs["bn2_g"], np.float32),
                         np.asarray(inputs["bn2_b"], np.float32)], 1),
        "ebn1": np.stack([np.asarray(inputs["ebn1_g"], np.float32),
                          np.asarray(inputs["ebn1_b"], np.float32)], 1),
        "ebn2": np.stack([np.asarray(inputs["ebn2_g"], np.float32),
                          np.asarray(inputs["ebn2_b"], np.float32)], 1),
        "gbn1": np.concatenate(
            [np.asarray(inputs["gbn1_g"], np.float32).reshape(2, 128).T,
             np.asarray(inputs["gbn1_b"], np.float32).reshape(2, 128).T], 1),
        "gbn2": np.concatenate(
            [np.asarray(inputs["gbn2_g"], np.float32).reshape(4, 128).T,
             np.asarray(inputs["gbn2_b"], np.float32).reshape(4, 128).T], 1),
        "pbn": np.concatenate(
            [np.asarray(inputs["pbn_g"], np.float32).reshape(4, 128).T,
             np.asarray(inputs["pbn_b"], np.float32).reshape(4, 128).T], 1),
        "ident": np.eye(128, dtype=np.float32),
        "neghalf": np.full((128, 128), -0.5, np.float32),
        "neghalfb": np.full((128, 128), -0.5, np.float32),
        "repmat": _repmat(),
    }
    in_maps = []
    for b in range(B):
        m = dict(common)
        m["xT"] = np.ascontiguousarray(pc[b].T)
        m["catb"] = np.ascontiguousarray(
            -0.2 * catb_all[cat_ids[b]].reshape(1, 512))
        in_maps.append(m)
    return in_maps


def _repmat():
    r = np.zeros((16, 128), np.float32)
    for m in range(128):
        r[m % 16, m] = 1.0
    return r


def kernel(**inputs):
    if "nc" not in _CACHE:
        _CACHE["nc"] = build_bass()
    nc = _CACHE["nc"]
    in_maps = host_prep(inputs)
    res = run_bass_kernel_spmd(nc, in_maps, list(range(NCORES)))
    outs = [res.results[b]["out"] for b in range(NCORES)]
    return np.stack(outs).astype(np.float32)


if __name__ == "__main__":
    import pickle
    inputs, expected = pickle.load(open("/tmp/inp.pkl", "rb"))
    got = kernel(**inputs)
    rel = np.linalg.norm(got - expected) / np.linalg.norm(expected)
    print("rel:", rel)

